# revision 34
# baseline (speedup 1.0000x reference)
"""Trainium2 Bass kernel for nn_CrossCategoryLoss.

loss(row) = sum_t relu(log_a[A_t] + log_b[B_t] - c_t)
  with c_t = log_g[G_t] (pos) or log(1 - exp(log_g[G_t])) (not).

Rewrites used (all per-row, exact in fp32 up to rounding):
  log_a[i] = alpha[i] - lsa,  lsa = ln(sum_j exp(alpha[j]))  (no max-sub
  needed: inputs are N(0,1), |x| < ~6, exp is safe in fp32)
  log(1-exp(log_g[k])) = ln(sum_g - exp(gamma[k])) - lsg
  term_t = relu(alpha[A] + beta[B] - q_t + S),  S = lsg - lsa - lsb
         = relu(p_AB - q'_t)   with p_AB = alpha[A]+beta[B], q'_t = q_t - S
    q_t = gamma[G]  (pos)  or  ln(sum_g - exp(gamma[G]))  (not)

Sharding: pure data-parallel over 8 cores; each core handles B/8 rows.
Per-core layout: rows viewed as [128 partitions, 4096 rows], tiles of
R rows per partition.
"""

import numpy as np

import concourse.bass as bass
import concourse.bacc as bacc
import concourse.mybir as mybir
from concourse.tile import TileContext
from concourse.bass_utils import run_bass_kernel_spmd

N_CORES = 8
B = 4194304
B_CORE = B // N_CORES          # 524288 rows per core
P = 128                        # partitions
ROWS_PER_PART = B_CORE // P    # 4096
R = 512                        # rows per partition per tile
N_TILES = ROWS_PER_PART // R   # 8

F32 = mybir.dt.float32
AX = mybir.AxisListType
AF = mybir.ActivationFunctionType
OP = mybir.AluOpType

# (alpha_idx, beta_idx, gamma_idx, is_not) - 36 constraint terms.
_TERMS = [
    (0, 4, 4, 0), (0, 4, 1, 1), (0, 4, 2, 1),
    (0, 6, 4, 0), (0, 6, 1, 1), (0, 6, 2, 1),
    (1, 5, 5, 0), (1, 5, 0, 1), (1, 5, 2, 1),
    (1, 6, 5, 0), (1, 6, 0, 1), (1, 6, 2, 1),
    (2, 4, 4, 0), (2, 4, 1, 1), (2, 4, 2, 1),
    (2, 5, 5, 0), (2, 5, 0, 1), (2, 5, 2, 1),
    (2, 6, 6, 0), (2, 7, 7, 0), (2, 7, 2, 1),
    (4, 0, 4, 0), (4, 0, 1, 1), (4, 0, 2, 1),
    (4, 2, 4, 0), (4, 2, 1, 1), (4, 2, 2, 1),
    (5, 1, 5, 0), (5, 1, 0, 1), (5, 1, 2, 1),
    (5, 2, 5, 0), (5, 2, 0, 1), (5, 2, 2, 1),
    (6, 2, 6, 0), (7, 2, 7, 0), (7, 2, 2, 1),
]

# Group terms by (a, b) pair, preserving first-appearance order.
_PAIRS: list[tuple[int, int]] = []
_PAIR_TERMS: dict[tuple[int, int], list[tuple[int, int]]] = {}
for _a, _b, _g, _n in _TERMS:
    if (_a, _b) not in _PAIR_TERMS:
        _PAIRS.append((_a, _b))
        _PAIR_TERMS[(_a, _b)] = []
    _PAIR_TERMS[(_a, _b)].append((_g, _n))

_NOT_GS = sorted({g for _, _, g, n in _TERMS if n})      # [0, 1, 2]
_POS_GS = sorted({g for _, _, g, n in _TERMS if not n})  # [4, 5, 6, 7]

# fp16 for the term stage: 16-bit dtype unlocks the DVE 2x_1P perf mode on
# tensor_tensor / tensor_scalar ops (fp32 is capped at 1x). Simulated error
# vs fp64 reference: norm-rel ~5e-4, absmax/scale ~6e-4. Stage 1 (exp sums,
# logs, S) stays fp32.
TERM_DT = mybir.dt.float16

# Pair slots in the P tile, ordered so every q-group's pair set is a
# contiguous slot range (enables one batched subtract per q-group):
_PAIR_SLOTS = [
    (0, 4), (0, 6), (2, 4), (4, 0), (4, 2),      # X1: q-triple {g4, w1, w2}
    (1, 5), (1, 6), (2, 5), (5, 1), (5, 2),      # X2: q-triple {g5, w0, w2}
    (2, 7), (7, 2), (2, 6), (6, 2),              # tail
]
# (q_key, pair_slot_range, d_slot_start): D[d0:d0+n] = P[p0:p1] - q
# Emission order: ScalarE's relu share (slots 12:36) is produced FIRST so
# its relu overlaps DVE's remaining subtracts; DVE's share (0:12) last.
_QGROUPS_ACT = [
    ((2, 1), 0, 12, 12),   # w2:  D[12:24] = P[0:12] - q_w2
    ((5, 0), 5, 10, 24),   # g5:  D[24:29] = P[5:10] - q_g5
    ((0, 1), 5, 10, 29),   # w0:  D[29:34] = P[5:10] - q_w0
    ((7, 0), 10, 12, 34),  # g7:  D[34:36] = P[10:12]- q_g7
]
_QGROUPS_DVE = [
    ((4, 0), 0, 5, 0),     # g4:  D[0:5]   = P[0:5]  - q_g4
    ((1, 1), 0, 5, 5),     # w1:  D[5:10]  = P[0:5]  - q_w1
    ((6, 0), 12, 14, 10),  # g6:  D[10:12] = P[12:14]- q_g6
]


def _bcast_mid(ap, n):
    """[P, R] access pattern -> [P, n, R] with a zero-stride middle dim."""
    a = ap[:, :]
    return bass.AP(tensor=a.tensor, offset=a.offset,
                   ap=[a.ap[0], [0, n], a.ap[1]])


def _swap_free(ap):
    """View a [P, K, R] tile iterated as [P, R, K] (same memory)."""
    a = ap[:, :, :]
    return bass.AP(tensor=a.tensor, offset=a.offset,
                   ap=[a.ap[0], a.ap[2], a.ap[1]])


def build_kernel(reps: int = 1) -> bass.Bass:
    nc = bacc.Bacc("TRN2", target_bir_lowering=False, debug=False,
                   num_devices=N_CORES)

    a_d = nc.dram_tensor("alpha_logits", [B_CORE, 8], F32, kind="ExternalInput")
    b_d = nc.dram_tensor("beta_logits", [B_CORE, 8], F32, kind="ExternalInput")
    g_d = nc.dram_tensor("gamma_logits", [B_CORE, 8], F32, kind="ExternalInput")
    o_d = nc.dram_tensor("loss", [B_CORE], F32, kind="ExternalOutput")

    a_v = a_d[:].rearrange("(p n) k -> p n k", p=P)
    b_v = b_d[:].rearrange("(p n) k -> p n k", p=P)
    g_v = g_d[:].rearrange("(p n) k -> p n k", p=P)
    o_v = o_d[:].rearrange("(p n) -> p n", p=P)

    with TileContext(nc) as tc:
        import contextlib
        rep_loop = tc.For_i(0, reps, 1) if reps > 1 else contextlib.nullcontext()
        with (
            rep_loop,
            tc.tile_pool(name="io", bufs=2) as io,
            tc.tile_pool(name="etmp", bufs=1) as etmp,
            tc.tile_pool(name="work", bufs=1) as work,
            tc.tile_pool(name="qpool", bufs=1) as qpool,
            tc.tile_pool(name="accp", bufs=1) as accp,
            tc.tile_pool(name="outp", bufs=2) as outp,
        ):
            for j in range(N_TILES):
                sl = slice(j * R, (j + 1) * R)

                a_t = io.tile([P, R, 8], F32, tag="a")
                b_t = io.tile([P, R, 8], F32, tag="b")
                g_t = io.tile([P, R, 8], F32, tag="g")
                nc.sync.dma_start(out=a_t, in_=a_v[:, sl, :])
                nc.sync.dma_start(out=b_t, in_=b_v[:, sl, :])
                nc.sync.dma_start(out=g_t, in_=g_v[:, sl, :])

                # --- stage 1: softmax denominators & logs (fp32) ---
                sums32 = {}
                eg_t = None
                for name, x_t in (("a", a_t), ("b", b_t), ("g", g_t)):
                    e_t = etmp.tile([P, R, 8], F32, tag="e", name=f"e{name}_{j}")
                    nc.scalar.activation(out=e_t, in_=x_t, func=AF.Exp)
                    s_t = work.tile([P, R], F32, tag="s" + name)
                    nc.vector.reduce_sum(out=s_t, in_=e_t, axis=AX.X)
                    sums32[name] = s_t
                    if name == "g":
                        eg_t = e_t
                sg_t = sums32["g"]

                # S = lsg - lsa - lsb = ln(sum_g) - ln(sum_a * sum_b)
                sab = work.tile([P, R], F32, tag="sab")
                nc.vector.tensor_mul(sab, sums32["a"], sums32["b"])
                lsab = work.tile([P, R], F32, tag="lsab")
                nc.scalar.activation(out=lsab, in_=sab, func=AF.Ln)
                lsg = work.tile([P, R], F32, tag="lsg")
                nc.scalar.activation(out=lsg, in_=sg_t, func=AF.Ln)
                s_t = work.tile([P, R], F32, tag="S")
                nc.vector.tensor_sub(s_t, lsg, lsab)

                # q' tiles: pos g: q' = gamma[g] - S ; not g: q' = ln(sum_g - e_g[g]) - S
                # Written in TERM_DT (fp16): halves DVE time of the term
                # stage via 2x_1P mode; error ~5e-4 rel (simulated).
                q = {}
                for gidx in _POS_GS:
                    qt = qpool.tile([P, R], TERM_DT, tag=f"qp{gidx}")
                    nc.vector.tensor_sub(qt, g_t[:, :, gidx], s_t)
                    q[(gidx, 0)] = qt
                for gidx in _NOT_GS:
                    wp = work.tile([P, R], F32, tag="wpre")
                    nc.vector.tensor_sub(wp, sg_t, eg_t[:, :, gidx])
                    wl = work.tile([P, R], F32, tag="wlog")
                    nc.scalar.activation(out=wl, in_=wp, func=AF.Ln)
                    qt = qpool.tile([P, R], TERM_DT, tag=f"qn{gidx}")
                    nc.vector.tensor_sub(qt, wl, s_t)
                    q[(gidx, 1)] = qt

                # --- stage 2: 36 terms, all fp16 2x/4x DVE ---
                # P tile: 14 pair sums (fp32 in -> fp16 out, 1x).
                p16 = work.tile([P, 14, R], TERM_DT, tag="p16")
                for i, (ai, bi) in enumerate(_PAIR_SLOTS):
                    nc.vector.tensor_add(p16[:, i, :],
                                         a_t[:, :, ai], b_t[:, :, bi])

                # D tile: one batched subtract per q-group (fp16 TT, 2x).
                # ScalarE's relu share is emitted as soon as its slots are
                # written, overlapping DVE's remaining subtracts.
                d36 = work.tile([P, 36, R], TERM_DT, tag="d36")
                for qkey, p0, p1, d0 in _QGROUPS_ACT:
                    n = p1 - p0
                    nc.vector.tensor_sub(
                        d36[:, d0:d0 + n, :], p16[:, p0:p1, :],
                        _bcast_mid(q[qkey], n),
                    )
                nc.scalar.activation(out=d36[:, 12:36, :], in_=d36[:, 12:36, :],
                                     func=AF.Relu)
                for qkey, p0, p1, d0 in _QGROUPS_DVE:
                    n = p1 - p0
                    nc.vector.tensor_sub(
                        d36[:, d0:d0 + n, :], p16[:, p0:p1, :],
                        _bcast_mid(q[qkey], n),
                    )
                nc.vector.tensor_scalar_max(d36[:, 0:12, :], d36[:, 0:12, :], 0.0)
                # Fold DVE's third first (no wait on ACT), then ACT's share.
                nc.vector.tensor_add(d36[:, 0:6, :], d36[:, 0:6, :],
                                     d36[:, 6:12, :])
                nc.vector.tensor_add(d36[:, 12:24, :], d36[:, 12:24, :],
                                     d36[:, 24:36, :])
                nc.vector.tensor_add(d36[:, 12:18, :], d36[:, 12:18, :],
                                     d36[:, 18:24, :])
                nc.vector.tensor_add(d36[:, 0:6, :], d36[:, 0:6, :],
                                     d36[:, 12:18, :])
                nc.vector.tensor_add(d36[:, 0:3, :], d36[:, 0:3, :],
                                     d36[:, 3:6, :])
                nc.vector.tensor_add(d36[:, 0, :], d36[:, 0, :], d36[:, 1, :])
                loss_t = outp.tile([P, R], F32, tag="loss")
                nc.vector.tensor_add(loss_t, d36[:, 0, :], d36[:, 2, :])
                nc.sync.dma_start(out=o_v[:, sl], in_=loss_t)

    nc.compile()
    return nc


_NC_CACHE = None


def _get_nc():
    global _NC_CACHE
    if _NC_CACHE is None:
        _NC_CACHE = build_kernel()
    return _NC_CACHE


def kernel(alpha_logits, beta_logits, gamma_logits, _trace=False):
    nc = _get_nc()
    in_maps = []
    for c in range(N_CORES):
        sl = slice(c * B_CORE, (c + 1) * B_CORE)
        in_maps.append({
            "alpha_logits": np.ascontiguousarray(alpha_logits[sl]),
            "beta_logits": np.ascontiguousarray(beta_logits[sl]),
            "gamma_logits": np.ascontiguousarray(gamma_logits[sl]),
        })
    res = run_bass_kernel_spmd(nc, in_maps, core_ids=list(range(N_CORES)),
                               trace=_trace)
    out = np.concatenate([r["loss"] for r in res.results])
    if _trace:
        kernel.last_result = res
    return out


# revision 35
# speedup vs baseline: 1.2741x; 1.2741x over previous
"""Trainium2 Bass kernel for nn_CrossCategoryLoss.

loss(row) = sum_t relu(log_a[A_t] + log_b[B_t] - c_t)
  with c_t = log_g[G_t] (pos) or log(1 - exp(log_g[G_t])) (not).

Rewrites used (all per-row, exact in fp32 up to rounding):
  log_a[i] = alpha[i] - lsa,  lsa = ln(sum_j exp(alpha[j]))  (no max-sub
  needed: inputs are N(0,1), |x| < ~6, exp is safe in fp32)
  log(1-exp(log_g[k])) = ln(sum_g - exp(gamma[k])) - lsg
  term_t = relu(alpha[A] + beta[B] - q_t + S),  S = lsg - lsa - lsb
         = relu(p_AB - q'_t)   with p_AB = alpha[A]+beta[B], q'_t = q_t - S
    q_t = gamma[G]  (pos)  or  ln(sum_g - exp(gamma[G]))  (not)

Sharding: pure data-parallel over 8 cores; each core handles B/8 rows.
Per-core layout: rows viewed as [128 partitions, 4096 rows], tiles of
R rows per partition.
"""

import numpy as np

import concourse.bass as bass
import concourse.bacc as bacc
import concourse.mybir as mybir
from concourse.tile import TileContext
from concourse.bass_utils import run_bass_kernel_spmd

N_CORES = 8
B = 4194304
B_CORE = B // N_CORES          # 524288 rows per core
P = 128                        # partitions
ROWS_PER_PART = B_CORE // P    # 4096
R = 512                        # rows per partition per tile
N_TILES = ROWS_PER_PART // R   # 8

F32 = mybir.dt.float32
AX = mybir.AxisListType
AF = mybir.ActivationFunctionType
OP = mybir.AluOpType

# (alpha_idx, beta_idx, gamma_idx, is_not) - 36 constraint terms.
_TERMS = [
    (0, 4, 4, 0), (0, 4, 1, 1), (0, 4, 2, 1),
    (0, 6, 4, 0), (0, 6, 1, 1), (0, 6, 2, 1),
    (1, 5, 5, 0), (1, 5, 0, 1), (1, 5, 2, 1),
    (1, 6, 5, 0), (1, 6, 0, 1), (1, 6, 2, 1),
    (2, 4, 4, 0), (2, 4, 1, 1), (2, 4, 2, 1),
    (2, 5, 5, 0), (2, 5, 0, 1), (2, 5, 2, 1),
    (2, 6, 6, 0), (2, 7, 7, 0), (2, 7, 2, 1),
    (4, 0, 4, 0), (4, 0, 1, 1), (4, 0, 2, 1),
    (4, 2, 4, 0), (4, 2, 1, 1), (4, 2, 2, 1),
    (5, 1, 5, 0), (5, 1, 0, 1), (5, 1, 2, 1),
    (5, 2, 5, 0), (5, 2, 0, 1), (5, 2, 2, 1),
    (6, 2, 6, 0), (7, 2, 7, 0), (7, 2, 2, 1),
]

# Group terms by (a, b) pair, preserving first-appearance order.
_PAIRS: list[tuple[int, int]] = []
_PAIR_TERMS: dict[tuple[int, int], list[tuple[int, int]]] = {}
for _a, _b, _g, _n in _TERMS:
    if (_a, _b) not in _PAIR_TERMS:
        _PAIRS.append((_a, _b))
        _PAIR_TERMS[(_a, _b)] = []
    _PAIR_TERMS[(_a, _b)].append((_g, _n))

_NOT_GS = sorted({g for _, _, g, n in _TERMS if n})      # [0, 1, 2]
_POS_GS = sorted({g for _, _, g, n in _TERMS if not n})  # [4, 5, 6, 7]

# fp16 for the term stage: 16-bit dtype unlocks the DVE 2x_1P perf mode on
# tensor_tensor / tensor_scalar ops (fp32 is capped at 1x). Simulated error
# vs fp64 reference: norm-rel ~5e-4, absmax/scale ~6e-4. Stage 1 (exp sums,
# logs, S) stays fp32.
TERM_DT = mybir.dt.float16

# Pair slots in the P tile, ordered so every q-group's pair set is a
# contiguous slot range (enables one batched subtract per q-group):
_PAIR_SLOTS = [
    (0, 4), (0, 6), (2, 4), (4, 0), (4, 2),      # X1: q-triple {g4, w1, w2}
    (1, 5), (1, 6), (2, 5), (5, 1), (5, 2),      # X2: q-triple {g5, w0, w2}
    (2, 7), (7, 2), (2, 6), (6, 2),              # tail
]
# (q_key, pair_slot_range, d_slot_start): D[d0:d0+n] = P[p0:p1] - q
_QGROUPS = [
    ((4, 0), 0, 5, 0),     # g4:  D[0:5]   = P[0:5]  - q_g4
    ((1, 1), 0, 5, 5),     # w1:  D[5:10]  = P[0:5]  - q_w1
    ((2, 1), 0, 12, 10),   # w2:  D[10:22] = P[0:12] - q_w2
    ((5, 0), 5, 10, 22),   # g5:  D[22:27] = P[5:10] - q_g5
    ((0, 1), 5, 10, 27),   # w0:  D[27:32] = P[5:10] - q_w0
    ((7, 0), 10, 12, 32),  # g7:  D[32:34] = P[10:12]- q_g7
    ((6, 0), 12, 14, 34),  # g6:  D[34:36] = P[12:14]- q_g6
]


def _bcast_mid(ap, n):
    """[P, R] access pattern -> [P, n, R] with a zero-stride middle dim."""
    a = ap[:, :]
    return bass.AP(tensor=a.tensor, offset=a.offset,
                   ap=[a.ap[0], [0, n], a.ap[1]])


def _swap_free(ap):
    """View a [P, K, R] tile iterated as [P, R, K] (same memory)."""
    a = ap[:, :, :]
    return bass.AP(tensor=a.tensor, offset=a.offset,
                   ap=[a.ap[0], a.ap[2], a.ap[1]])


def build_kernel(reps: int = 1) -> bass.Bass:
    nc = bacc.Bacc("TRN2", target_bir_lowering=False, debug=False,
                   num_devices=N_CORES)

    a_d = nc.dram_tensor("alpha_logits", [B_CORE, 8], F32, kind="ExternalInput")
    b_d = nc.dram_tensor("beta_logits", [B_CORE, 8], F32, kind="ExternalInput")
    g_d = nc.dram_tensor("gamma_logits", [B_CORE, 8], F32, kind="ExternalInput")
    o_d = nc.dram_tensor("loss", [B_CORE], F32, kind="ExternalOutput")

    a_v = a_d[:].rearrange("(p n) k -> p n k", p=P)
    b_v = b_d[:].rearrange("(p n) k -> p n k", p=P)
    g_v = g_d[:].rearrange("(p n) k -> p n k", p=P)
    o_v = o_d[:].rearrange("(p n) -> p n", p=P)

    with TileContext(nc) as tc:
        import contextlib
        rep_loop = tc.For_i(0, reps, 1) if reps > 1 else contextlib.nullcontext()
        with (
            rep_loop,
            tc.tile_pool(name="io", bufs=2) as io,
            tc.tile_pool(name="etmp", bufs=1) as etmp,
            tc.tile_pool(name="work", bufs=1) as work,
            tc.tile_pool(name="qpool", bufs=1) as qpool,
            tc.tile_pool(name="accp", bufs=1) as accp,
            tc.tile_pool(name="outp", bufs=2) as outp,
        ):
            for j in range(N_TILES):
                sl = slice(j * R, (j + 1) * R)

                a_t = io.tile([P, R, 8], F32, tag="a")
                b_t = io.tile([P, R, 8], F32, tag="b")
                g_t = io.tile([P, R, 8], F32, tag="g")
                nc.sync.dma_start(out=a_t, in_=a_v[:, sl, :])
                nc.sync.dma_start(out=b_t, in_=b_v[:, sl, :])
                nc.sync.dma_start(out=g_t, in_=g_v[:, sl, :])

                # --- stage 1: softmax denominators & logs (fp32) ---
                sums32 = {}
                eg_t = None
                for name, x_t in (("a", a_t), ("b", b_t), ("g", g_t)):
                    e_t = etmp.tile([P, R, 8], F32, tag="e", name=f"e{name}_{j}")
                    nc.scalar.activation(out=e_t, in_=x_t, func=AF.Exp)
                    s_t = work.tile([P, R], F32, tag="s" + name)
                    nc.vector.reduce_sum(out=s_t, in_=e_t, axis=AX.X)
                    sums32[name] = s_t
                    if name == "g":
                        eg_t = e_t
                sg_t = sums32["g"]

                # S = lsg - lsa - lsb = ln(sum_g) - ln(sum_a * sum_b)
                sab = work.tile([P, R], F32, tag="sab")
                nc.vector.tensor_mul(sab, sums32["a"], sums32["b"])
                lsab = work.tile([P, R], F32, tag="lsab")
                nc.scalar.activation(out=lsab, in_=sab, func=AF.Ln)
                lsg = work.tile([P, R], F32, tag="lsg")
                nc.scalar.activation(out=lsg, in_=sg_t, func=AF.Ln)
                s_t = work.tile([P, R], F32, tag="S")
                nc.vector.tensor_sub(s_t, lsg, lsab)

                # q' tiles: pos g: q' = gamma[g] - S ; not g: q' = ln(sum_g - e_g[g]) - S
                # Written in TERM_DT (fp16): halves DVE time of the term
                # stage via 2x_1P mode; error ~5e-4 rel (simulated).
                q = {}
                for gidx in _POS_GS:
                    qt = qpool.tile([P, R], TERM_DT, tag=f"qp{gidx}")
                    nc.vector.tensor_sub(qt, g_t[:, :, gidx], s_t)
                    q[(gidx, 0)] = qt
                for gidx in _NOT_GS:
                    wp = work.tile([P, R], F32, tag="wpre")
                    nc.vector.tensor_sub(wp, sg_t, eg_t[:, :, gidx])
                    wl = work.tile([P, R], F32, tag="wlog")
                    nc.scalar.activation(out=wl, in_=wp, func=AF.Ln)
                    qt = qpool.tile([P, R], TERM_DT, tag=f"qn{gidx}")
                    nc.vector.tensor_sub(qt, wl, s_t)
                    q[(gidx, 1)] = qt

                # --- stage 2: 36 terms, all fp16 2x/4x DVE ---
                # P tile: 14 pair sums (fp32 in -> fp16 out, 1x).
                p16 = work.tile([P, 14, R], TERM_DT, tag="p16")
                for i, (ai, bi) in enumerate(_PAIR_SLOTS):
                    nc.vector.tensor_add(p16[:, i, :],
                                         a_t[:, :, ai], b_t[:, :, bi])

                # D tile: one batched subtract per q-group (fp16 TT, 2x).
                d36 = work.tile([P, 36, R], TERM_DT, tag="d36")
                for qkey, p0, p1, d0 in _QGROUPS:
                    n = p1 - p0
                    nc.vector.tensor_sub(
                        d36[:, d0:d0 + n, :], p16[:, p0:p1, :],
                        _bcast_mid(q[qkey], n),
                    )

                # relu in place, split across engines: DVE does the first
                # half (tensor_scalar 4x), ScalarE the second half (it is
                # otherwise idle while DVE is the bottleneck). Then pairwise
                # tree sum (fp16 TT adds, 2x), final level in fp32.
                nc.vector.tensor_scalar_max(d36[:, 0:12, :], d36[:, 0:12, :], 0.0)
                nc.scalar.activation(out=d36[:, 12:36, :], in_=d36[:, 12:36, :],
                                     func=AF.Relu)
                # Fold DVE's third first (no wait on ACT), then ACT's share.
                nc.vector.tensor_add(d36[:, 0:6, :], d36[:, 0:6, :],
                                     d36[:, 6:12, :])
                nc.vector.tensor_add(d36[:, 12:24, :], d36[:, 12:24, :],
                                     d36[:, 24:36, :])
                nc.vector.tensor_add(d36[:, 12:18, :], d36[:, 12:18, :],
                                     d36[:, 18:24, :])
                nc.vector.tensor_add(d36[:, 0:6, :], d36[:, 0:6, :],
                                     d36[:, 12:18, :])
                nc.vector.tensor_add(d36[:, 0:3, :], d36[:, 0:3, :],
                                     d36[:, 3:6, :])
                nc.vector.tensor_add(d36[:, 0, :], d36[:, 0, :], d36[:, 1, :])
                loss_t = outp.tile([P, R], F32, tag="loss")
                nc.vector.tensor_add(loss_t, d36[:, 0, :], d36[:, 2, :])
                nc.sync.dma_start(out=o_v[:, sl], in_=loss_t)

    nc.compile()
    return nc


_NC_CACHE = None


def _get_nc():
    global _NC_CACHE
    if _NC_CACHE is None:
        _NC_CACHE = build_kernel()
    return _NC_CACHE


def kernel(alpha_logits, beta_logits, gamma_logits, _trace=False):
    nc = _get_nc()
    in_maps = []
    for c in range(N_CORES):
        sl = slice(c * B_CORE, (c + 1) * B_CORE)
        in_maps.append({
            "alpha_logits": np.ascontiguousarray(alpha_logits[sl]),
            "beta_logits": np.ascontiguousarray(beta_logits[sl]),
            "gamma_logits": np.ascontiguousarray(gamma_logits[sl]),
        })
    res = run_bass_kernel_spmd(nc, in_maps, core_ids=list(range(N_CORES)),
                               trace=_trace)
    out = np.concatenate([r["loss"] for r in res.results])
    if _trace:
        kernel.last_result = res
    return out
